# revision 42
# baseline (speedup 1.0000x reference)
"""DenseCRFLoss on 8 Trainium2 NeuronCores.

Math: loss = -W/N * sum_k s_k^T K s_k per image, K[p,q] = exp(-0.5*||f_p-f_q||^2),
f = (x/50, y/50, rgb/15) on the 64x64 downsampled image, P = 4096 pixels.

Scheme (everything folded into ONE matmul + elementwise exp + row-sum):
  With d = s0 - s1 the class weight is W[p,q] = s0p*s0q + s1p*s1q
  = (1 + dp*dq)/2, strictly positive, so
      sum_pq K*W = sum_pq exp( f_p.f_q - |f_p|^2/2 - |f_q|^2/2
                               + ln(1 + dp*dq) - ln 2 ).
  ln(1+x) is expanded as sum_k c_k x^k (weighted LS fit on the empirical
  range of x = dp*dq), each term c_k dp^k * dq^k is one extra bf16
  contraction row of the single G matmul; the -0.5|f|^2 - ln2/2 biases ride
  along as 4 more rows; features use a bf16 hi/lo split (hh+hl+lh).  The
  diagonal 128x128 subtiles of the 8 diagonal chunks need weight W/2: an
  extra -ln2 on exactly those columns is injected by 4 indicator row pairs
  (stat row jj = 1 on p-slice jj, mov row jj = -ln2 on q in [128jj,128jj+128),
  straddle quads only).  Only the upper triangle is computed; the total is
  doubled on the host (the half-weighted diagonal makes that exact).

  The PSUM tile holds x = g/32.  Two consumers split the column stream:
   - ScalarE: exp(32x) (free scale) with free accum_out row sums,
   - VectorE: two custom DVE ops: p = 1 + x + c0 x^2 + x^3(c1 + c2 x)
     (Horner, 8 ALU ops) then relu(p)^32 + accum (7 ops) -- a polynomial
     exp that runs at 1 elem/lane/cycle per pass.
  Row-sums land in one [128, ntiles] accumulator, DMA'd out once; the host
  adds them up.  Work: 18 quads/core (36 512x512 upper-tri chunks per image,
  2 cores per image); straddle (diagonal) quads pack as 1280 columns.
"""

import numpy as np
import ml_dtypes

WEIGHT = 2e-9
SIGMA_RGB = 15.0
SIGMA_XY = 100.0
SCALE = 0.5
LN2 = float(np.log(2.0))

NQ = 18                 # quads per core
STRADDLE_SLOTS = (14, 15, 16, 17)
SERIES_K = 8            # ln(1+x) series depth
# F rows: spatial hh (2) + color hh/hl/lh (9); bias 4; series K; diag 4
NROWS = 11 + 4 + SERIES_K + 4
INV = 1.0 / 32.0        # PSUM holds g/32 (5 squarings on the DVE path)

# exp-poly for the DVE path: p = 1 + x + C[0] x^2 + C[1] x^3 + C[2] x^4 on
# [-0.672, 0.02] (validated: max rel 3.5e-4 -> 1.1e-2 after ^32 worst-case,
# 5.2e-3 measured on E > 1e-8); c2 < 0 makes p -> -inf below the range so
# relu self-clamps.  exp(g) for g < -21.5 is approximated by ~0.
EXP_POLY = (0.49109172, 0.12904958, -0.0005)

# straddle-quad packed block layout: block j covers chunk cols [128j:512];
# packed into [0:1280] as j0,j1,j3,j2 (no matmul crosses a PSUM bank).
_S_ORDER = (0, 1, 3, 2)

_bf16 = ml_dtypes.bfloat16
_PROGRAM_CACHE = {}
_OPS_CACHE = {}


def _exp_ops():
    """Register the two custom DVE ops (idempotent)."""
    if _OPS_CACHE:
        return _OPS_CACHE["a"], _OPS_CACHE["b"]
    import concourse.dve_ops as dve_ops
    from concourse.dve_spec import (
        Spec, Src0, C0, C1, C2, Zero, One, AluOp, sq, relu, lower,
    )
    from concourse.dve_ops import DveOp
    from concourse.dve_uop import DveOpSpec

    def make(name, spec):
        if name in dve_ops._SUB_OPCODE_FOR_NAME:
            op = next(o for o in dve_ops.OPS if o.name == name)
            return op
        shas = {}
        for ver in ("v3", "v4"):
            try:
                shas[ver] = DveOpSpec(
                    name=name, opcode=0, uops=lower(spec, ver=ver), rd1_en=True
                ).sha(ver)
            except Exception:  # noqa: BLE001  (v4 may reject; TRN2 uses v3)
                pass
        op = DveOp(name, spec, False, uops_sha=shas)
        dve_ops.OPS.append(op)
        dve_ops.CUSTOM_DVE_SPECS[name] = spec
        dve_ops._SUB_OPCODE_FOR_NAME[name] = (
            max(dve_ops._SUB_OPCODE_FOR_NAME.values()) + 1
        )
        assert max(dve_ops._SUB_OPCODE_FOR_NAME.values()) < 0x20
        return op

    def ref_a(in0, in1, s0, s1, imm2):
        x = in0.astype(np.float32)
        p = (np.float32(1.0) + x) + (x * x) * (
            np.float32(s0) + x * (np.float32(s1) + np.float32(imm2) * x))
        return p.astype(np.float32)

    def ref_b(in0, in1, s0, s1, imm2):
        x = np.maximum(in0.astype(np.float32), 0.0)
        for _ in range(5):
            x = x * x
        return x, x.sum(axis=-1, dtype=np.float32).reshape(-1, 1)

    inner = C0 + Src0 * (C1 + C2 * Src0)
    expa = make("EXPA_ANT", Spec(
        body=(One + Src0) + sq(Src0) * inner, reference=ref_a))
    expb = make("EXPB_ANT", Spec(
        body=sq(sq(sq(sq(sq(relu(Src0)))))), reference=ref_b,
        accum=AluOp.ADD, accum_init=Zero))
    _OPS_CACHE["a"], _OPS_CACHE["b"] = expa, expb
    return expa, expb


def _segments():
    """Per-core column stream: list of (quad, j, mov_lo, width) in order,
    split nowhere (splitting at bank/tile boundaries happens at emit)."""
    segs = []
    for i in range(NQ):
        if i in STRADDLE_SLOTS:
            for j in _S_ORDER:
                segs.append((i, j, 128 * j, 512 - 128 * j))
        else:
            for j in range(4):
                segs.append((i, j, 0, 512))
    assert sum(s[3] for s in segs) == 33792
    return segs


def _tile_plan():
    """Sequence of (consumer, width) consuming the 33792-col stream.
    'A' = ScalarE exp tile (<=1536 = 3 PSUM banks, double-buffered),
    'D' = VectorE poly-exp tile (<=1024 = 2 banks, single-buffered).
    Greedy balance on measured per-tile costs (ns), including engine start
    offsets and the DVE per-tile refill stall; small leading tiles for an
    early start, small trailing tiles for a short tail."""
    total = 33792

    def a_cost(w):
        return (w + 437) / 1.2

    def d_cost(w):
        return (2 * w + 178) / 0.96 + 160.0

    t_a, t_d = 3500.0, 3900.0  # observed first-available times
    plan = []
    used = 0
    for c, w in (("A", 512), ("D", 512), ("A", 1024), ("D", 1024)):
        plan.append((c, w))
        used += w
        if c == "A":
            t_a += a_cost(w)
        else:
            t_d += d_cost(w)
    # reserve small closing tiles
    tail = [("A", 512), ("D", 512)]
    used += 1024
    while total - used >= 1536:
        if t_a + a_cost(1536) <= t_d + d_cost(1024) or total - used < 1024:
            plan.append(("A", 1536))
            t_a += a_cost(1536)
            used += 1536
        else:
            plan.append(("D", 1024))
            t_d += d_cost(1024)
            used += 1024
    rem = total - used
    remainder = []
    while rem > 0:
        w = min(rem, 1024)
        remainder.append(("A" if t_a <= t_d else "D", w))
        if remainder[-1][0] == "A":
            t_a += a_cost(w)
        else:
            t_d += d_cost(w)
        rem -= w
    # small tiles go at the very end: a short activation cannot hide the
    # next refill, so mid-stream small tiles would stall the consumer
    plan += remainder + tail
    assert sum(w for _, w in plan) == total
    return plan


def _build_program():
    import concourse.bacc as bacc
    import concourse.tile as tile
    from concourse import mybir

    expa, expb = _exp_ops()

    nc = bacc.Bacc("TRN2", target_bir_lowering=False)
    dt = mybir.dt

    plan = _tile_plan()
    n_acc = len(plan)

    feat = nc.dram_tensor("feat", [NROWS, 2, NQ, 512], dt.bfloat16,
                          kind="ExternalInput")
    out = nc.dram_tensor("out", [128, n_acc], dt.float32, kind="ExternalOutput")

    segs = _segments()

    with tile.TileContext(nc) as tc:
        with (
            tc.tile_pool(name="consts", bufs=1) as consts,
            tc.tile_pool(name="apsum", bufs=2, space="PSUM") as apool,
            tc.tile_pool(name="dpsum", bufs=1, space="PSUM") as dpool,
        ):
            feat_sb = consts.tile([NROWS, 2, NQ, 512], dt.bfloat16)
            stat_sb = feat_sb[:, 0]
            mov_sb = feat_sb[:, 1]
            acc = consts.tile([128, n_acc], dt.float32)
            scr = consts.tile([128, 1024], dt.float32)
            junk = consts.tile([128, 128], dt.bfloat16)

            # PE warm-up fodder; DVE is idle this early
            nc.vector.memset(junk, 0.0)

            # graded input chunks; straddle quads (14-17) stream last
            chunks = [(0, 1), (1, 2), (2, 3), (3, 5), (5, 8), (8, 12), (12, 18)]
            for lo, hi in chunks:
                nc.sync.dma_start(out=feat_sb[:, :, lo:hi, :],
                                  in_=feat[:, :, lo:hi, :])

            # warmup landing zone + ~2.5us of warm-up matmuls so the PE
            # p-state ramps while the input DMAs land
            warm = apool.tile([128, 1536], dt.float32, name="a_t")
            for _w in range(10):
                nc.tensor.matmul(out=warm[:, 0:128], lhsT=junk[:, 0:128],
                                 rhs=junk[:, 0:128], start=True, stop=True)

            # pre-resolve each tile's matmul pieces (tile-offset, quad, j, lo, w)
            seg_iter = iter(segs)
            cur = None
            tile_pieces = []
            for cons, tw in plan:
                pieces = []
                off = 0
                while off < tw:
                    if cur is None:
                        s = next(seg_iter)
                        cur = [s[0], s[1], s[2], s[3], 0]
                    i, j, lo, width, done = cur
                    room = 512 - (off % 512)
                    w = min(min(room, tw - off), width - done)
                    pieces.append((off, i, j, lo + done, w))
                    cur[4] += w
                    if cur[4] == width:
                        cur = None
                    off += w
                tile_pieces.append(pieces)
            assert cur is None

            # predicted-eligibility emission order: PE executes its queue
            # in-order (head-of-line blocking), so refills must be emitted in
            # the order their WAR hazards clear.
            t_eng = {"A": 3500.0, "D": 3900.0}
            pool_prev = {"A": [None, None], "D": [None]}
            events = []  # (pred_time, seq, kind, tile_idx)
            for t, (cons, tw) in enumerate(plan):
                slot = (len([1 for c, _ in plan[:t] if c == cons])
                        % len(pool_prev[cons]))
                war = pool_prev[cons][slot]
                elig = 0.0 if war is None else war + 200.0
                if cons == "A":
                    dur = (tw + 437) / 1.2
                else:
                    dur = (2 * tw + 178) / 0.96
                start = max(t_eng[cons], elig + tw / 2.4 + 200.0)
                end = start + dur
                t_eng[cons] = end
                pool_prev[cons][slot] = end
                events.append((elig, 2 * t, "mm", t))
                events.append((start, 2 * t + 1, "cons", t))
            events.sort()

            tiles_psum = {}
            for _pt, _seq, kind, t in events:
                cons, tw = plan[t]
                if kind == "mm":
                    if cons == "A":
                        g = apool.tile([128, 1536], dt.float32, name="a_t")
                    else:
                        g = dpool.tile([128, 1024], dt.float32, name="d_t")
                    tiles_psum[t] = g
                    for off, i, j, mlo, w in tile_pieces[t]:
                        nc.tensor.matmul(
                            out=g[:, off:off + w],
                            lhsT=stat_sb[:, i, 128 * j:128 * (j + 1)],
                            rhs=mov_sb[:, i, mlo:mlo + w],
                            start=True, stop=True,
                        )
                else:
                    g = tiles_psum.pop(t)
                    if cons == "A":
                        nc.scalar.activation(
                            out=g[:, 0:tw], in_=g[:, 0:tw],
                            func=mybir.ActivationFunctionType.Exp,
                            scale=32.0,
                            accum_out=acc[:, t:t + 1],
                        )
                    else:
                        nc.vector._custom_dve(
                            expa, out=scr[:, 0:tw], in0=g[:, 0:tw],
                            s0=float(EXP_POLY[0]), s1=float(EXP_POLY[1]),
                            imm2=float(EXP_POLY[2]))
                        nc.vector._custom_dve(
                            expb, out=scr[:, 0:tw], in0=scr[:, 0:tw],
                            accum_out=acc[:, t:t + 1])

            nc.sync.dma_start(out=out[:, :], in_=acc[:, :])

    nc.compile()
    return nc


def _get_program():
    if "p" not in _PROGRAM_CACHE:
        _PROGRAM_CACHE["p"] = _build_program()
    return _PROGRAM_CACHE["p"]


def _quad_assignment():
    """Per-image quad lists for the two cores sharing an image; straddle
    (diagonal) chunks land on STRADDLE_SLOTS."""
    full = [(c, q) for c in range(8) for q in range(c)]  # 28 off-diag chunks
    stra = [(c, c) for c in range(8)]                    # 8 diagonal chunks

    def arrange(fulls, stras):
        fi, si = iter(fulls), iter(stras)
        return [next(si) if s in STRADDLE_SLOTS else next(fi)
                for s in range(NQ)]

    even = arrange(full[0::2], stra[0:4])
    odd = arrange(full[1::2], stra[4:8])
    return even, odd


def _fit_series(d):
    """Weighted LS fit of ln(1+x) ~ sum_{k=1..K} c_k x^k on the empirical
    range of x = dp*dq, weight W(x) = (1+x)/2."""
    m = float(np.abs(d).max())
    xmax = min(m * m, 0.9999)
    x = np.linspace(-xmax, xmax, 20001)
    w = (1.0 + x) / 2.0
    A = np.stack([x ** k for k in range(1, SERIES_K + 1)], axis=1)
    c, *_ = np.linalg.lstsq(A * w[:, None], np.log1p(x) * w, rcond=None)
    return c


def _prepare_inputs(images, segmentations):
    N = images.shape[0]
    assert images.shape == (4, 3, 128, 128)
    assert segmentations.shape == (4, 2, 128, 128)

    # nearest resize (scale 0.5) == stride-2 subsample
    img = images[:, :, ::2, ::2].astype(np.float64)
    # bilinear resize (scale 0.5, align_corners=False) == 2x2 average,
    # mirroring the reference's fp32 evaluation order
    s = segmentations.astype(np.float32)
    t = s[:, :, 0::2, :] * np.float32(0.5) + s[:, :, 1::2, :] * np.float32(0.5)
    seg = t[:, :, :, 0::2] * np.float32(0.5) + t[:, :, :, 1::2] * np.float32(0.5)
    seg = seg.reshape(N, 2, 4096).astype(np.float64)
    d = seg[:, 0] - seg[:, 1]                       # [N, P]

    sxy = SIGMA_XY * SCALE
    yy, xx = np.meshgrid(np.arange(64.0), np.arange(64.0), indexing="ij")
    pos = np.stack([xx, yy], 0) / sxy
    feats = np.concatenate(
        [np.broadcast_to(pos[None], (N, 2, 64, 64)), img / SIGMA_RGB], axis=1)
    F = feats.reshape(N, 5, 4096)
    F = F - F.mean(axis=2, keepdims=True)           # shrink |f| (exactness-free)
    b = -0.5 * (F * F).sum(axis=1) - 0.5 * LN2      # [N, P]

    cs = _fit_series(d)

    def split(x):
        h = x.astype(_bf16).astype(np.float64)
        l = (x - h).astype(_bf16).astype(np.float64)
        return h, l

    Fh, Fl = split(F)
    Bh, Bl = split(b)
    ones = np.ones((N, 1, 4096))

    def r(x):
        return x.astype(_bf16).astype(np.float64)

    # paired contraction rows (stat_row[r] * mov_row[r] summed over r):
    # spatial dims (0:2) need no hi/lo split (|f| < 0.7); color dims (2:5)
    # use the hh/hl/lh cross; bias rows Bh+Bl on both sides; then series.
    Ph, Ch, Cl = Fh[:, 0:2], Fh[:, 2:5], Fl[:, 2:5]
    stat_rows = [Ph, Ch, Ch, Cl, Bh[:, None], Bl[:, None], ones, ones]
    mov_rows = [Ph, Ch, Cl, Ch, ones, ones, Bh[:, None], Bl[:, None]]
    for k in range(1, SERIES_K + 1):
        stat_rows.append(r(cs[k - 1] * d[:, None] ** k)[..., :])
        mov_rows.append(r(d[:, None] ** k))
    STAT = np.concatenate([r(a) if a is not ones else a for a in stat_rows],
                          axis=1)
    MOV = np.concatenate([r(a) if a is not ones else a for a in mov_rows],
                         axis=1)
    assert STAT.shape[1] == NROWS - 4

    # scale the stat side by 1/32 (exact in bf16): PSUM = g/32
    STAT = (STAT.astype(_bf16).astype(np.float64) * INV).astype(_bf16)
    MOV = MOV.astype(_bf16)

    # diag-indicator rows: stat side = indicator of the p 128-slice (x 1/32),
    # mov side = -ln2 on the matching q 128-slice (straddle quads only)
    ind_stat = np.zeros((4, 512), _bf16)
    ind_mov = np.zeros((4, 512), _bf16)
    for jj in range(4):
        ind_stat[jj, 128 * jj:128 * (jj + 1)] = _bf16(INV)
        ind_mov[jj, 128 * jj:128 * (jj + 1)] = _bf16(-LN2)

    even, odd = _quad_assignment()

    in_maps = []
    for core in range(8):
        im = core // 2
        quads = even if core % 2 == 0 else odd
        feat_arr = np.zeros((NROWS, 2, NQ, 512), _bf16)
        for slot, (c, q) in enumerate(quads):
            feat_arr[:NROWS - 4, 0, slot, :] = STAT[im][:, 512 * q:512 * (q + 1)]
            feat_arr[:NROWS - 4, 1, slot, :] = MOV[im][:, 512 * c:512 * (c + 1)]
            feat_arr[NROWS - 4:, 0, slot, :] = ind_stat
            if slot in STRADDLE_SLOTS:
                feat_arr[NROWS - 4:, 1, slot, :] = ind_mov
        in_maps.append({"feat": np.ascontiguousarray(feat_arr)})
    return in_maps


def _combine(outs, n_images=4):
    total = sum(float(o["out"].sum(dtype=np.float64)) for o in outs)
    loss = -WEIGHT * 2.0 * total / n_images
    return np.array([loss], dtype=np.float32)


def kernel(images, segmentations):
    from concourse.bass_utils import run_bass_kernel_spmd

    in_maps = _prepare_inputs(np.asarray(images), np.asarray(segmentations))
    nc = _get_program()
    last_err = None
    for _attempt in range(3):  # the NRT backend occasionally fails transiently
        try:
            res = run_bass_kernel_spmd(nc, in_maps, core_ids=list(range(8)))
            return _combine(res.results)
        except Exception as e:  # noqa: BLE001
            last_err = e
    raise last_err
